# revision 1
# baseline (speedup 1.0000x reference)
"""Contrastive loss (SimCLR-style) on 8 Trainium2 NeuronCores.

Full inputs in, full output out.  Each core owns a 1024-row block of
feats; the host passes each core a rolled copy of feats so the block is
always local rows 0..1023 (static self-mask diagonal, identical SPMD
program on every core).

Symmetry split: exp(cos/T) is symmetric, so core x only computes its
block rows against local column blocks 0..4 (cols 0..5119).  Row sums
over the remaining column blocks 5..7 are recovered from *column* sums
of blocks (x, x+1..x+3), which other cores' rows need by symmetry:
column sums are accumulated on the PE with a ones-stationary matmul and
shipped to the host, which adds them into the right rows.  The device
normalizes rows, transposes to bf16 nfT on the PE, matmuls block rows
against columns, masks self, and row-sums exp(cos/T) with the scalar
engine's fused accumulate.  Positive-pair cosines come from
host-gathered partner rows.  Host: assemble S, logsumexp, mean.
"""

from contextlib import ExitStack

import numpy as np

N, D, NCORES = 8192, 128, 8
BLK = N // NCORES            # 1024 rows per core
TPB = BLK // 128             # 8 M-tiles (of 128 rows) per core
NT = N // 128                # 64 row tiles total
TEMP = 0.07
EPS = 1e-8
MASK_SUB = 30.0              # cos - 30 -> exp((cos-30)/T) == 0 in fp32
CHUNK = 512                  # matmul moving-operand columns
QCOLS = 1024                 # psum tile columns (2 banks)
NQ = 5                       # direct column blocks per core (cols 0..5119)
CSBLKS = 3                   # column-sum blocks (local col blocks 1..3)

_CACHE = {}
LAST_RESULT = None


def _emit(tc, xr, pr, ident_d, eyeneg_d, s_out, pos_out, cs_out, rep=0):
    import concourse.mybir as mybir

    nc = tc.nc
    f32 = mybir.dt.float32
    bf16 = mybir.dt.bfloat16
    AF = mybir.ActivationFunctionType
    AX = mybir.AxisListType.X

    with ExitStack() as ctx:
        singles = ctx.enter_context(tc.tile_pool(name=f"singles{rep}", bufs=1))
        work = ctx.enter_context(tc.tile_pool(name=f"work{rep}", bufs=3))

        xbig = singles.tile([128, NT * D], f32, tag="xbig")      # rolled X, row-major
        nfT = singles.tile([128, N], bf16, tag="nfT")            # normalized X, transposed
        nfblk = singles.tile([128, BLK], f32, tag="nfblk")       # nf rows 0..1023, row-major
        pbig = singles.tile([128, TPB * D], f32, tag="pbig")     # partner rows, row-major
        ss = singles.tile([128, NT], f32, tag="ss")
        nrm = singles.tile([128, NT], f32, tag="nrm")
        rall = singles.tile([128, NT], f32, tag="rall")
        ssp = singles.tile([128, TPB], f32, tag="ssp")
        nrmp = singles.tile([128, TPB], f32, tag="nrmp")
        rp = singles.tile([128, TPB], f32, tag="rp")
        posv = singles.tile([128, TPB], f32, tag="posv")
        sv = singles.tile([128, TPB], f32, tag="sv")
        parts = singles.tile([128, TPB * NQ], f32, tag="parts")
        ident = singles.tile([128, 128], f32, tag="ident")
        eyeneg = singles.tile([128, 128], f32, tag="eyeneg")
        ones = singles.tile([128, 128], bf16, tag="ones")
        colacc = singles.tile([128, CSBLKS * QCOLS], f32, tag="colacc")

        # ---- loads ----
        nc.sync.dma_start(out=ident[:], in_=ident_d)
        nc.sync.dma_start(out=eyeneg[:], in_=eyeneg_d)
        nc.vector.memset(ones[:], 1.0)
        xr3 = xr.rearrange("(t p) d -> p t d", p=128)
        xbig3 = xbig[:].rearrange("p (t d) -> p t d", d=D)
        GRP = 8                                   # tiles per load chunk
        for g in range(NT // GRP):
            nc.sync.dma_start(out=xbig3[:, g * GRP:(g + 1) * GRP, :],
                              in_=xr3[:, g * GRP:(g + 1) * GRP, :])
        nc.sync.dma_start(
            out=pbig[:].rearrange("p (t d) -> p t d", d=D),
            in_=pr.rearrange("(t p) d -> p t d", p=128),
        )

        # ---- phase A: row norms -> 1/max(||x||, eps), in pipelined batches ----
        # 1/sqrt(ss) computed as exp(-0.5*ln(ss)): Ln and Exp live in the same
        # ACT table set, so the whole kernel needs exactly one table load.
        # (tensor_tensor_reduce wedges the device on this runtime; use mul+reduce)
        NB = 16                                   # tiles per norm batch
        for b in range(NT // NB):
            for t in range(b * NB, (b + 1) * NB):
                j = work.tile([128, D], f32, tag="junk")
                nc.vector.tensor_mul(j[:], xbig[:, t * D:(t + 1) * D], xbig[:, t * D:(t + 1) * D])
                nc.vector.reduce_sum(out=ss[:, t:t + 1], in_=j[:], axis=AX)
            bs = slice(b * NB, (b + 1) * NB)
            nc.vector.tensor_scalar_max(ss[:, bs], ss[:, bs], EPS * EPS)
            nc.scalar.activation(nrm[:, bs], ss[:, bs], AF.Ln)
            nc.scalar.activation(rall[:, bs], nrm[:, bs], AF.Exp, scale=-0.5)

        # ---- phase A2: normalize + transpose into nfT (bf16) ----
        with tc.tile_pool(name=f"tpsum{rep}", bufs=2, space="PSUM") as tpsum:
            for t in range(NT):
                if t < TPB:
                    nf_ap = nfblk[:, t * D:(t + 1) * D]
                else:
                    nf_t = work.tile([128, D], f32, tag="nf")
                    nf_ap = nf_t[:]
                nc.vector.tensor_scalar_mul(nf_ap, xbig[:, t * D:(t + 1) * D], rall[:, t:t + 1])
                pt = tpsum.tile([128, 128], f32, tag="tp")
                nc.tensor.transpose(pt[:], nf_ap, ident[:])
                nc.vector.tensor_copy(nfT[:, t * D:(t + 1) * D], pt[:])

        # ---- phase C: similarity chunks + exp row-sums + column sums ----
        # q outer so only one column block's PSUM accumulators are live.
        # High priority: the exp pipeline is the kernel bottleneck, so its
        # matmuls/mask-adds should win engine picks over leftover phase-A work.
        with (
            tc.tile_pool(name=f"mpsum{rep}", bufs=2, space="PSUM") as mpsum,
            tc.tile_pool(name=f"cpsum{rep}", bufs=2, space="PSUM") as cpsum,
            tc.tile_pool(name=f"escratch{rep}", bufs=3) as esp,
            tc.high_priority(),
        ):
            for q in range(NQ):
                do_cs = 1 <= q <= CSBLKS
                if do_cs:
                    cs0 = cpsum.tile([128, CHUNK], f32, tag="cs0")
                    cs1 = cpsum.tile([128, CHUNK], f32, tag="cs1")
                for m in range(TPB):
                    lhsT = nfT[:, m * 128:(m + 1) * 128]
                    pt = mpsum.tile([128, QCOLS], f32, tag="mp")
                    for jj in range(QCOLS // CHUNK):
                        n0 = q * QCOLS + jj * CHUNK
                        nc.tensor.matmul(
                            pt[:, jj * CHUNK:(jj + 1) * CHUNK],
                            lhsT, nfT[:, n0:n0 + CHUNK], start=True, stop=True,
                        )
                    if q == 0:
                        # self column of local row m*128+p is m*128+p (rolled input)
                        nc.vector.tensor_add(
                            pt[:, m * 128:(m + 1) * 128],
                            pt[:, m * 128:(m + 1) * 128], eyeneg[:],
                        )
                    e = esp.tile([128, QCOLS], bf16, tag="e")
                    nc.scalar.activation(
                        e[:], pt[:], AF.Exp, scale=1.0 / TEMP,
                        accum_out=parts[:, m * NQ + q:m * NQ + q + 1],
                    )
                    if do_cs:
                        # column sums of exp accumulated across the 8 M-tiles
                        nc.tensor.matmul(cs0[:], ones[:], e[:, :CHUNK],
                                         start=(m == 0), stop=(m == TPB - 1),
                                         skip_group_check=True)
                        nc.tensor.matmul(cs1[:], ones[:], e[:, CHUNK:],
                                         start=(m == 0), stop=(m == TPB - 1),
                                         skip_group_check=True)
                if do_cs:
                    k = q - 1
                    nc.vector.tensor_copy(colacc[:, k * QCOLS:k * QCOLS + CHUNK], cs0[:])
                    nc.vector.tensor_copy(colacc[:, k * QCOLS + CHUNK:(k + 1) * QCOLS], cs1[:])
        # ---- phase B: positive-pair cosines (fills DVE/ACT gaps during C) ----
        for t in range(TPB):
            j = work.tile([128, D], f32, tag="junk")
            nc.vector.tensor_mul(j[:], pbig[:, t * D:(t + 1) * D], pbig[:, t * D:(t + 1) * D])
            nc.vector.reduce_sum(out=ssp[:, t:t + 1], in_=j[:], axis=AX)
        nc.vector.tensor_scalar_max(ssp[:], ssp[:], EPS * EPS)
        nc.scalar.activation(nrmp[:], ssp[:], AF.Ln)
        nc.scalar.activation(rp[:], nrmp[:], AF.Exp, scale=-0.5)
        for t in range(TPB):
            npf = work.tile([128, D], f32, tag="nf")
            nc.vector.tensor_scalar_mul(npf[:], pbig[:, t * D:(t + 1) * D], rp[:, t:t + 1])
            j = work.tile([128, D], f32, tag="junk")
            nc.vector.tensor_mul(j[:], nfblk[:, t * D:(t + 1) * D], npf[:])
            nc.vector.reduce_sum(out=posv[:, t:t + 1], in_=j[:], axis=AX)
        nc.sync.dma_start(out=pos_out, in_=posv[:])

        for m in range(TPB):
            nc.vector.reduce_sum(out=sv[:, m:m + 1], in_=parts[:, m * NQ:(m + 1) * NQ], axis=AX)
        nc.sync.dma_start(out=s_out, in_=sv[:])
        nc.sync.dma_start(out=cs_out, in_=colacc[0:1, :])


def _build_nc(repeats=1):
    import concourse.tile as tile
    import concourse.mybir as mybir
    from concourse import bacc

    f32 = mybir.dt.float32
    nc = bacc.Bacc(
        "TRN2", target_bir_lowering=False, debug=False,
        enable_asserts=False, num_devices=NCORES,
    )
    xr_h = nc.dram_tensor("xr", [N, D], f32, kind="ExternalInput")
    pr_h = nc.dram_tensor("partner", [BLK, D], f32, kind="ExternalInput")
    id_h = nc.dram_tensor("ident", [128, 128], f32, kind="ExternalInput")
    en_h = nc.dram_tensor("eyeneg", [128, 128], f32, kind="ExternalInput")
    s_h = nc.dram_tensor("s_out", [128, TPB], f32, kind="ExternalOutput")
    p_h = nc.dram_tensor("pos_out", [128, TPB], f32, kind="ExternalOutput")
    c_h = nc.dram_tensor("cs_out", [1, CSBLKS * QCOLS], f32, kind="ExternalOutput")

    with tile.TileContext(nc, trace_sim=False) as tc:
        for rep in range(repeats):
            _emit(tc, xr_h.ap(), pr_h.ap(), id_h.ap(), en_h.ap(),
                  s_h.ap(), p_h.ap(), c_h.ap(), rep=rep)
    nc.compile()
    return nc


def get_nc(repeats=1):
    key = ("nc", repeats)
    if key not in _CACHE:
        _CACHE[key] = _build_nc(repeats)
    return _CACHE[key]


def make_in_maps(feats, label):
    feats = np.ascontiguousarray(np.asarray(feats, dtype=np.float32))
    label = np.asarray(label)
    pos_idx = np.argmax(label, axis=1)
    partner = feats[pos_idx]
    ident = np.eye(128, dtype=np.float32)
    eyeneg = (-MASK_SUB * np.eye(128)).astype(np.float32)
    in_maps = []
    for c in range(NCORES):
        xr = np.concatenate([feats[c * BLK:], feats[:c * BLK]], axis=0)
        in_maps.append({
            "xr": np.ascontiguousarray(xr),
            "partner": np.ascontiguousarray(partner[c * BLK:(c + 1) * BLK]),
            "ident": ident,
            "eyeneg": eyeneg,
        })
    return in_maps


def finish(results):
    """Host epilogue: assemble full row sums from direct row partials and
    symmetric column partials, then logsumexp and mean."""
    S = np.zeros(N, dtype=np.float64)
    pos = np.zeros(N, dtype=np.float64)
    for x in range(NCORES):
        sv = results[x]["s_out"].astype(np.float64)       # [128, TPB]
        S[x * BLK:(x + 1) * BLK] += sv.T.reshape(-1)      # local rows in order
        pv = results[x]["pos_out"].astype(np.float64)
        pos[x * BLK:(x + 1) * BLK] = pv.T.reshape(-1)
        cs = results[x]["cs_out"].astype(np.float64).reshape(CSBLKS, BLK)
        for k in range(1, CSBLKS + 1):
            tgt = ((x + k) % NCORES) * BLK                # rows of block x+k
            S[tgt:tgt + BLK] += cs[k - 1]
    lse = np.log(S)
    loss = (lse - pos / TEMP).mean()
    return np.array(loss, dtype=np.float32)


def kernel(feats, label, _trace=False, _repeats=1):
    global LAST_RESULT
    from concourse.bass_utils import run_bass_kernel_spmd

    nc = get_nc(_repeats)
    in_maps = make_in_maps(feats, label)
    res = run_bass_kernel_spmd(nc, in_maps, list(range(NCORES)), trace=_trace)
    LAST_RESULT = res
    return finish(res.results)



# revision 8
# speedup vs baseline: 2.8034x; 2.8034x over previous
"""Contrastive loss (SimCLR-style) on 8 Trainium2 NeuronCores.

Full inputs in, full output out.  Host pre-normalizes feats (f32), takes
the positive-pair cosines on the host (8192x128 dot products - trivial
next to the 256MB label argmax already done there), and ships each core
a rolled, transposed bf16 copy nfT = roll(nf).T of the 5120 columns the
core actually touches (1.25MB).  The device then only does the heavy
part: the N/8 x 5N/8 block of similarities, exp, and row/column sums.

Symmetry split: exp(cos/T) is symmetric, so core x only computes its
1024 rows against local column blocks 0..4 (cols 0..5119).  Row sums
over the remaining column blocks 5..7 are recovered from *column* sums
of blocks (x, x+1..x+3), which other cores' rows need by symmetry.
Per M-tile the 5120 columns run as PSUM spans of 2048/2048/1024 so the
ACT exp (the bottleneck engine) pays its 352-cycle ramp 24x, not 40x.
Column sums accumulate on the otherwise-idle DVE in bf16; the final
partition reduction is 24 skinny stationary=acc matmuls that write a
transposed [128, 24] tile so the output DMA uses all 128 partitions
instead of a slow single-partition 12KB line.  Host: assemble S from
row partials + shipped column sums, logsumexp, mean.
"""

from contextlib import ExitStack

import numpy as np

N, D, NCORES = 8192, 128, 8
BLK = N // NCORES            # 1024 rows per core
TPB = BLK // 128             # 8 M-tiles (of 128 rows) per core
TEMP = 0.07
EPS = 1e-8
MASK_SUB = 30.0              # cos - 30 -> exp((cos-30)/T) == 0 in fp32
QCOLS = 1024                 # one column block
MMCHUNK = 512                # matmul moving-operand columns
NQ = 5                       # direct column blocks per core (cols 0..5119)
CSBLKS = 3                   # column-sum blocks (local col blocks 1..3)
NCOLS = NQ * QCOLS           # 5120 columns shipped per core
SPANS = ((0, 2048, 0), (2048, 2048, 1), (4096, 1024, 2))  # (col0, width, slot)

_CACHE = {}
LAST_RESULT = None


def _emit(tc, nfT_d, masks_d, s_out, cs_out, rep=0):
    import concourse.mybir as mybir

    nc = tc.nc
    f32 = mybir.dt.float32
    bf16 = mybir.dt.bfloat16
    AF = mybir.ActivationFunctionType
    AX = mybir.AxisListType.X
    NSP = len(SPANS)

    with ExitStack() as ctx:
        singles = ctx.enter_context(tc.tile_pool(name=f"singles{rep}", bufs=1))

        nfT = singles.tile([128, NCOLS], bf16, tag="nfT")    # normalized X^T
        masks = singles.tile([128, 256], bf16, tag="masks")  # [ident | -30*ident]
        ones = singles.tile([128, 512], bf16, tag="ones")
        parts = singles.tile([128, TPB * 4], f32, tag="parts")
        sv = singles.tile([128, TPB], f32, tag="sv")
        acc = singles.tile([128, CSBLKS * QCOLS], bf16, tag="acc")
        csg = singles.tile([128, CSBLKS * TPB], f32, tag="csg")
        wact = singles.tile([128, 8], f32, tag="wact")

        # ---- ACT warmup, overlapped with the input DMA window ----
        # The first ACTIVATE pays the ~2.7us exp table load; trigger it at
        # t=0 on a tiny memset tile so the load hides under the nfT DMA.
        # (No PE warmup: its PSUM slot reuse makes the first real matmul
        # wait for every dummy one, which costs more than the cold clock.)
        nc.vector.memset(wact[:], 0.0)
        nc.scalar.activation(wact[:], wact[:], AF.Exp)
        nc.vector.memset(ones[:], 1.0)

        # ---- loads (chunk0 gates the first span; masks right behind) ----
        nc.sync.dma_start(out=nfT[:, 0:QCOLS], in_=nfT_d[:, 0:QCOLS])
        nc.sync.dma_start(out=masks[:], in_=masks_d)
        for a in range(QCOLS, NCOLS, QCOLS):
            nc.sync.dma_start(out=nfT[:, a:a + QCOLS], in_=nfT_d[:, a:a + QCOLS])

        # ---- similarity spans + exp row-sums + column sums ----
        # m=0 runs 1024-wide spans so the first exp is gated by only two
        # matmuls on the freshly-DMAed chunk0, not four.
        SPANS0 = ((0, 1024, 0), (1024, 1024, 1), (2048, 2048, 2), (4096, 1024, 3))
        with (
            tc.tile_pool(name=f"mpsum{rep}", bufs=2, space="PSUM") as mpsum,
            tc.tile_pool(name=f"escratch{rep}", bufs=3) as esp,
            tc.high_priority(),
        ):
            for m in range(TPB):
                lhsT = nfT[:, m * 128:(m + 1) * 128]
                for c0, w, slot in (SPANS0 if m == 0 else SPANS):
                    pt = mpsum.tile([128, 2048], f32, tag="mp")
                    for jj in range(w // MMCHUNK):
                        nc.tensor.matmul(
                            pt[:, jj * MMCHUNK:(jj + 1) * MMCHUNK],
                            lhsT, nfT[:, c0 + jj * MMCHUNK:c0 + (jj + 1) * MMCHUNK],
                            start=True, stop=True,
                        )
                    if c0 == 0:
                        # self column of local row m*128+p is m*128+p (rolled
                        # input): accumulate (-30I)^T @ I onto the diagonal
                        # sub-block on the PE so exp never waits on another
                        # engine.
                        nc.tensor.matmul(
                            pt[:, m * 128:(m + 1) * 128],
                            masks[:, 128:256], masks[:, 0:128],
                            start=False, stop=True, skip_group_check=True,
                        )
                    e = esp.tile([128, 2048], bf16, tag="e")
                    nc.scalar.activation(
                        e[:, :w], pt[:, :w], AF.Exp, scale=1.0 / TEMP,
                        accum_out=parts[:, m * 4 + slot:m * 4 + slot + 1],
                    )
                    # column-sum accumulation (cols 1024..4095 only) on DVE
                    lo = max(c0, QCOLS)
                    hi = min(c0 + w, (CSBLKS + 1) * QCOLS)
                    if lo < hi:
                        a = acc[:, lo - QCOLS:hi - QCOLS]
                        eslice = e[:, lo - c0:hi - c0]
                        if m == 0:
                            nc.vector.tensor_copy(a, eslice)
                        else:
                            nc.vector.tensor_add(a, a, eslice)
                nsl = 4 if m == 0 else NSP
                nc.vector.reduce_sum(out=sv[:, m:m + 1],
                                     in_=parts[:, m * 4:m * 4 + nsl], axis=AX)

        # ---- tail: transpose-reduce column sums ----
        with tc.tile_pool(name=f"tpsum{rep}", bufs=1, space="PSUM") as tpsum:
            cspT = tpsum.tile([128, CSBLKS * TPB], f32, tag="cspT")
            for k in range(CSBLKS * TPB):
                # cspT[c, k] = sum_p acc[p, 128k + c]  (all output cols equal)
                nc.tensor.matmul(cspT[:, k:k + 1],
                                 acc[:, k * 128:(k + 1) * 128], ones[:, 0:1],
                                 start=True, stop=True)
            nc.vector.tensor_copy(csg[:], cspT[:])
        nc.sync.dma_start(out=s_out, in_=sv[:])
        nc.sync.dma_start(out=cs_out, in_=csg[:])


def _build_nc(repeats=1):
    import concourse.tile as tile
    import concourse.mybir as mybir
    from concourse import bacc

    f32 = mybir.dt.float32
    bf16 = mybir.dt.bfloat16
    nc = bacc.Bacc(
        "TRN2", target_bir_lowering=False, debug=False,
        enable_asserts=False, num_devices=NCORES,
    )
    nfT_h = nc.dram_tensor("nfT", [128, NCOLS], bf16, kind="ExternalInput")
    mk_h = nc.dram_tensor("masks", [128, 256], bf16, kind="ExternalInput")
    s_h = nc.dram_tensor("s_out", [128, TPB], f32, kind="ExternalOutput")
    c_h = nc.dram_tensor("cs_out", [128, CSBLKS * TPB], f32, kind="ExternalOutput")

    with tile.TileContext(nc, trace_sim=False) as tc:
        for rep in range(repeats):
            _emit(tc, nfT_h.ap(), mk_h.ap(), s_h.ap(), c_h.ap(), rep=rep)
    nc.compile()
    return nc


def get_nc(repeats=1):
    key = ("nc", repeats)
    if key not in _CACHE:
        _CACHE[key] = _build_nc(repeats)
    return _CACHE[key]


def _host_prep(feats, label):
    """Normalize on host, build per-core rolled nfT (bf16) and host pos."""
    import ml_dtypes

    feats = np.asarray(feats, dtype=np.float32)
    label = np.asarray(label)
    norms = np.sqrt((feats.astype(np.float64) ** 2).sum(axis=1))
    nf = feats / np.maximum(norms, EPS)[:, None].astype(np.float32)
    pos_idx = np.argmax(label, axis=1)
    pos = np.einsum("ij,ij->i", nf.astype(np.float64), nf[pos_idx].astype(np.float64))
    nfT_full = np.ascontiguousarray(nf.T.astype(ml_dtypes.bfloat16))  # [128, N]
    nfT2 = np.concatenate([nfT_full, nfT_full[:, :NCOLS]], axis=1)
    masks = np.concatenate([np.eye(128), -MASK_SUB * np.eye(128)],
                           axis=1).astype(ml_dtypes.bfloat16)
    in_maps = []
    for c in range(NCORES):
        in_maps.append({
            "nfT": np.ascontiguousarray(nfT2[:, c * BLK:c * BLK + NCOLS]),
            "masks": masks,
        })
    return in_maps, pos


def make_in_maps(feats, label):
    in_maps, _ = _host_prep(feats, label)
    return in_maps


def finish(results, pos):
    """Host epilogue: assemble full row sums from direct row partials and
    symmetric column partials, then logsumexp and mean."""
    S = np.zeros(N, dtype=np.float64)
    for x in range(NCORES):
        sv = results[x]["s_out"].astype(np.float64)       # [128, TPB]
        S[x * BLK:(x + 1) * BLK] += sv.T.reshape(-1)      # local rows in order
        # cs_out[c, k]: column sum of local column 128k+c (k = 8*(q-1)+sub)
        cs = results[x]["cs_out"].astype(np.float64).T.reshape(CSBLKS, BLK)
        for k in range(1, CSBLKS + 1):
            tgt = ((x + k) % NCORES) * BLK                # rows of block x+k
            S[tgt:tgt + BLK] += cs[k - 1]
    lse = np.log(S)
    loss = (lse - pos / TEMP).mean()
    return np.array(loss, dtype=np.float32)


def kernel(feats, label, _trace=False, _repeats=1):
    global LAST_RESULT
    from concourse.bass_utils import run_bass_kernel_spmd

    nc = get_nc(_repeats)
    in_maps, pos = _host_prep(feats, label)
    res = run_bass_kernel_spmd(nc, in_maps, list(range(NCORES)), trace=_trace)
    LAST_RESULT = res
    return finish(res.results, pos)
